# revision 42
# baseline (speedup 1.0000x reference)
"""ANFIS forward pass on 8 Trainium2 NeuronCores, pure data parallelism.

Math reformulation (per batch row b, rule r, input i, m = rule_indices[r,i]):
  log firing[b,r] = sum_i -0.5*((x_bi - c_im)/s_im)^2  (+ ln mask_r)
                  = sum_i A_ir*x_bi^2 + B_ir*x_bi + const_r
so firing comes from ONE matmul over features F=[x^2; x; 1] (K=33) with the
membership gather folded into host-precomputed weights.  The rule reduction
(firing_sum and sum_r firing*rule_out) is a second matmul contracting the 64
rules:  H_ext = firing @ [C | 1], then out = sigmoid((x_aug . H)/den) with
the reference's uniform-weight fallback selected where den <= 1e-12.

Device layout: features are uploaded pre-transposed [33, B] fp16 (host does
the transpose; fp16 is enough mantissa since the PE computes in FP22 anyway;
validated max rel err ~9e-5 end to end).  MM1 keeps the tiny weight matrix
stationary, loaded into two PE column-groups so even/odd 512-column chunks
stack into one [128,512] psum bank -> full-width exp on ScalarE.  MM2 uses
the firing tile as stationary (bf16 -> fast weight load) and streams the
small consequent matrix, producing H_ext directly in batch-on-partitions
layout for a cheap wide vector epilogue.
"""

import numpy as np

N_CORES = 8
B_FULL = 131072
BS = B_FULL // N_CORES          # 16384 rows per core
N_IN, N_MF, N_RULES = 16, 2, 64
KF = 2 * N_IN                   # 32 feature rows (x^2, x)
CH = 512                        # batch columns per MM1 matmul
# megachunk sizes
MCS = [2048] * 8
MCO = [sum(MCS[:i]) for i in range(len(MCS))]
N_MC = len(MCS)
XW = 18                         # xh row: 16 x + 1 one + 1 fallback value

_compiled = None


def _build_graph():
    from concourse import bacc, tile, mybir

    nc = bacc.Bacc()
    dt = mybir.dt
    Alu = mybir.AluOpType
    Act = mybir.ActivationFunctionType

    ft_ext = nc.declare_dram_parameter("ft", [KF, BS], dt.float16, isOutput=False)
    xh_ext = nc.declare_dram_parameter("xh", [128, (BS // 128) * XW], dt.float16,
                                       isOutput=False)
    wcl_ext = nc.declare_dram_parameter("wcl", [KF, 128], dt.float16,
                                        isOutput=False)
    wcr_ext = nc.declare_dram_parameter("wcr", [KF, 128], dt.float16,
                                        isOutput=False)
    w2_ext = nc.declare_dram_parameter("w2", [128, 36], dt.bfloat16, isOutput=False)
    cb_ext = nc.declare_dram_parameter("cb", [128, 1], dt.float32, isOutput=False)
    out_ext = nc.declare_dram_parameter("out", [128, BS // 128], dt.float32,
                                        isOutput=True)

    with tile.TileContext(nc) as tc:
        with (
            tc.tile_pool(name="const", bufs=1) as cpool,
            tc.tile_pool(name="feat", bufs=1) as fpool,
            tc.tile_pool(name="xha", bufs=1) as xpool,
            tc.tile_pool(name="fir", bufs=2) as firpool,
            tc.tile_pool(name="scratch", bufs=2) as spool,
            tc.tile_pool(name="stats", bufs=1) as statpool,
            tc.tile_pool(name="ps1", bufs=3, space="PSUM") as ps1pool,
            tc.tile_pool(name="ps2", bufs=2, space="PSUM") as ps2pool,
        ):
            # two stationaries [33,128]: logF weights in cols 0:64 / 64:128,
            # zeros elsewhere, so the even/odd chunk matmuls form one
            # accumulation group over the full [128,512] psum bank.
            wcL = cpool.tile([KF, 128], dt.float16)
            nc.gpsimd.dma_start(wcL[:], wcl_ext[:])
            wcR = cpool.tile([KF, 128], dt.float16)
            nc.gpsimd.dma_start(wcR[:], wcr_ext[:])
            w2 = cpool.tile([128, 36], dt.bfloat16)
            nc.gpsimd.dma_start(w2[:], w2_ext[:])
            cb = cpool.tile([128, 1], dt.float32)
            nc.scalar.dma_start(cb[:], cb_ext[:])

            nst = BS // 128
            num_all = statpool.tile([128, nst], dt.float32)
            den_all = statpool.tile([128, nst], dt.float32)
            fb_all = statpool.tile([128, nst], dt.float32)

            # ---- all input loads up front into distinct per-mc tiles:
            # single writer + single reader each => minimal sync waits,
            # and the DMA queues stream ahead of compute.
            feats, xhas = [], []
            for mc in range(N_MC):
                S, off = MCS[mc], MCO[mc]
                eng = nc.sync if mc % 2 == 0 else nc.gpsimd
                if mc == 0:
                    # four separate tiles so mm1(0) pipelines behind the load
                    qs = S // 4
                    quarters = []
                    for q in range(4):
                        fq = fpool.tile([KF, qs], dt.float16,
                                        name=f"feat{mc}q{q}")
                        eng.dma_start(fq[:],
                                      ft_ext[:, off + q * qs:off + (q + 1) * qs])
                        quarters.append(fq)
                    feats.append(quarters)
                else:
                    feat = fpool.tile([KF, S], dt.float16, name=f"feat{mc}")
                    eng.dma_start(feat[:], ft_ext[:, off:off + S])
                    feats.append(feat)
                xha = xpool.tile([128, (S // 128) * XW], dt.float16,
                                 name=f"xha{mc}")
                eng.dma_start(
                    xha[:], xh_ext[:, (off // 128) * XW:((off + S) // 128) * XW])
                xhas.append(xha)

            def emit_mm1(mc):
                feat = feats[mc]
                ps1 = ps1pool.tile([128, MCS[mc] // 2], dt.float32,
                                   name=f"ps1_{mc}", tag="ps1")

                def chunk(c):
                    if isinstance(feat, list):
                        return feat[c][:]
                    return feat[:, c * CH:(c + 1) * CH]

                for bank in range(MCS[mc] // 1024):
                    nc.tensor.matmul(
                        ps1[:, bank * CH:(bank + 1) * CH],
                        wcL[:], chunk(2 * bank),
                        start=True, stop=False,
                    )
                    nc.tensor.matmul(
                        ps1[:, bank * CH:(bank + 1) * CH],
                        wcR[:], chunk(2 * bank + 1),
                        start=False, stop=True,
                    )
                return ps1

            ps1_next = emit_mm1(0)
            for mc in range(N_MC):
                xha = xhas[mc]
                ps1 = ps1_next
                if mc + 1 < N_MC:
                    ps1_next = emit_mm1(mc + 1)

                S, off = MCS[mc], MCO[mc]
                nblk = S // 256
                # ---- exp over the whole psum tile -> firing (bf16)
                fir = firpool.tile([128, S // 2], dt.bfloat16, tag="fir")
                nc.scalar.activation(fir[:], ps1[:], Act.Exp, bias=cb[:])

                # ---- MM2: contract rules; firing slices are stationary
                ps2 = ps2pool.tile([128, nblk * 36], dt.float32, tag="ps2")
                for t in range(nblk):
                    nc.tensor.matmul(
                        ps2[:, t * 36:(t + 1) * 36],
                        fir[:, t * 128:(t + 1) * 128],
                        w2[:],
                        start=(t == 0), stop=(t == nblk - 1),
                    )

                # ---- epilogue: num = sum_j xaug_j * H_j ; den ; fb
                # ps2 block t cols: [H_e(0:17) | H_o(17:34) | den_e | den_o]
                # host stores xh tiles in block order (t4, tm, h) so all APs
                # are <=4D: [p, block t, half g, j]
                sc = off // 128
                scw = S // 128
                h_ap = ps2[:].rearrange("p (t f) -> p t f", t=nblk)[:, :, 0:34] \
                             .rearrange("p t (g j) -> p t g j", g=2)
                xh_ap = xha[:].rearrange("p (t g j) -> p t g j", t=nblk, g=2)
                prod = spool.tile([128, 8 * 2 * 17], dt.float32, tag="prod")
                prod_ap = prod[:, 0:nblk * 2 * 17] \
                    .rearrange("p (t g j) -> p t g j", t=nblk, g=2)
                nc.vector.tensor_tensor(prod_ap, h_ap,
                                        xh_ap[:, :, :, 0:17], Alu.mult)
                num_mc = num_all[:, sc:sc + scw] \
                    .rearrange("p (t g) -> p t g", t=nblk)
                nc.vector.tensor_reduce(num_mc, prod_ap,
                                        axis=mybir.AxisListType.X, op=Alu.add)
                den_src = ps2[:].rearrange("p (t f) -> p t f", t=nblk)[:, :, 34:36]
                nc.vector.tensor_copy(
                    den_all[:, sc:sc + scw]
                    .rearrange("p (t g) -> p t g", t=nblk), den_src)
                nc.gpsimd.tensor_copy(
                    fb_all[:, sc:sc + scw]
                    .rearrange("p (t g) -> p t g", t=nblk).unsqueeze(3),
                    xh_ap[:, :, :, 17:18])

            # ---- final: out = clip(sigmoid(select(den<=eps, fb, num/den)))
            # processed in halves so most of the serial chain overlaps the
            # last megachunks' compute
            dmax = statpool.tile([128, nst], dt.float32)
            rec = statpool.tile([128, nst], dt.float32)
            u = statpool.tile([128, nst], dt.float32)
            cond = statpool.tile([128, nst], dt.uint8)
            th = statpool.tile([128, nst], dt.float32)
            sig = statpool.tile([128, nst], dt.float32)
            outb = statpool.tile([128, nst], dt.float32)
            nc.vector.tensor_scalar_max(dmax[:], den_all[:], 1e-12)
            nc.vector.reciprocal(rec[:], dmax[:])
            nc.vector.tensor_tensor(u[:], num_all[:], rec[:], Alu.mult)
            nc.vector.tensor_scalar(cond[:], den_all[:], 1e-12, None,
                                    op0=Alu.is_le)
            nc.vector.copy_predicated(u[:], cond[:], fb_all[:])
            # sigmoid(u) = 0.5*tanh(u/2) + 0.5  (tanh shares the exp table set)
            nc.scalar.activation(th[:], u[:], Act.Tanh, scale=0.5)
            nc.vector.tensor_scalar(sig[:], th[:], 0.5, 0.5,
                                    op0=Alu.mult, op1=Alu.add)
            nc.vector.tensor_scalar(outb[:], sig[:], 1e-7, 1.0 - 1e-7,
                                    op0=Alu.max, op1=Alu.min)
            nc.sync.dma_start(out_ext[:], outb[:])

    nc.finalize()
    return nc


def _prepare(inputs):
    """Host-side weight folding + feature building. Returns per-core in_maps."""
    import ml_dtypes

    x = np.asarray(inputs["x"], np.float32)
    center = np.asarray(inputs["center"], np.float32)
    log_sigma = np.asarray(inputs["log_sigma"], np.float32)
    consequent = np.asarray(inputs["consequent"], np.float32)
    rule_idx = np.asarray(inputs["rule_indices"]).astype(np.int64)
    mask = np.asarray(inputs["active_mask"], np.float32)

    sigma = np.exp(log_sigma) + 1e-6
    inv_s2 = 1.0 / (sigma * sigma)                       # [I, M]
    ar = np.arange(N_IN)
    is2 = inv_s2[ar[None, :], rule_idx]                  # [R, I]
    c_ri = center[ar[None, :], rule_idx]                 # [R, I]
    A = -0.5 * is2                                        # x^2 coeff [R, I]
    Bc = is2 * c_ri                                       # x coeff   [R, I]
    const_r = np.sum(-0.5 * is2 * c_ri * c_ri, axis=1)    # [R]
    with np.errstate(divide="ignore"):
        lnm = np.where(mask > 0, np.log(np.maximum(mask, 1e-38)), -1e30)
    const_r = np.maximum(const_r + lnm, -1e30)

    wc = np.zeros((KF, 64), np.float32)
    wc[0:N_IN, :] = A.T
    wc[N_IN:2 * N_IN, :] = Bc.T
    cb = np.concatenate([const_r, const_r]).reshape(128, 1).astype(np.float32)

    # MM2 weights; block cols [H_e(0:17) | H_o(17:34) | den_e(34) | den_o(35)]
    w2 = np.zeros((128, 36), np.float32)
    w2[0:64, 0:17] = consequent
    w2[0:64, 34] = 1.0
    w2[64:128, 17:34] = consequent
    w2[64:128, 35] = 1.0

    # fallback: out_pre = x_aug . (C^T @ fbvec)
    fbvec = mask / max(float(mask.sum()), 1.0)
    vfb = consequent.T @ fbvec                            # [17]

    h16 = x.astype(np.float16)
    q16 = (x * x).astype(np.float16)
    fbv = (x @ vfb[:16] + vfb[16]).astype(np.float16)     # [B]

    in_maps = []
    for c in range(N_CORES):
        xs = slice(c * BS, (c + 1) * BS)
        ft = np.empty((KF, BS), np.float16)
        ft[0:N_IN, :] = q16[xs].T
        ft[N_IN:2 * N_IN, :] = h16[xs].T
        # xh rows: [x(16) | 1 | fb]; tiled to [128, (BS/128)*18]
        xa = np.empty((BS, XW), np.float16)
        xa[:, 0:16] = h16[xs]
        xa[:, 16] = np.float16(1.0)
        xa[:, 17] = fbv[xs]
        # tile order within each megachunk: (bank, tm, h); batch =
        # off + b*1024 + h*512 + tm*128 + p
        parts = []
        for S, off in zip(MCS, MCO):
            xt = xa[off:off + S].reshape(S // 1024, 2, 4, 128, XW)
            parts.append(xt.transpose(3, 0, 2, 1, 4).reshape(128, -1))
        xh = np.ascontiguousarray(np.concatenate(parts, axis=1))
        wcl = np.zeros((KF, 128), np.float32); wcl[:, 0:64] = wc
        wcr = np.zeros((KF, 128), np.float32); wcr[:, 64:128] = wc
        in_maps.append({
            "ft": ft,
            "xh": xh,
            "wcl": wcl.astype(np.float16),
            "wcr": wcr.astype(np.float16),
            "w2": w2.astype(ml_dtypes.bfloat16),
            "cb": cb,
        })
    return in_maps


def _unpermute(out_t):
    """out_t [128, BS/128]: per-mc cols ordered (bank, tm, h); batch =
    off + b*1024 + h*512 + tm*128 + p."""
    o = np.asarray(out_t, np.float32)
    res = np.empty(BS, np.float32)
    for S, off in zip(MCS, MCO):
        c0 = off // 128
        blk = o[:, c0:c0 + S // 128].reshape(128, S // 1024, 4, 2)
        res[off:off + S] = blk.transpose(1, 3, 2, 0).reshape(S)
    return res


def kernel(**inputs) -> np.ndarray:
    global _compiled
    from concourse.bass_utils import run_bass_kernel_spmd

    if _compiled is None:
        _compiled = _build_graph()
    in_maps = _prepare(inputs)
    res = run_bass_kernel_spmd(_compiled, in_maps, core_ids=list(range(N_CORES)))
    outs = [np.asarray(res.results[i]["out"], np.float32) for i in range(N_CORES)]
    return np.concatenate([_unpermute(o) for o in outs], axis=0)
